# revision 3
# baseline (speedup 1.0000x reference)
import numpy as np
import jax
import jax.numpy as jnp

# Problem constants (hardcoded; kernel.py must be self-contained)
B, C, H, W = 8, 256, 64, 64
HEADS, R, TOPK_FRAC = 8, 4, 0.25
DIM = C // HEADS
SCALE = DIM ** -0.5
N = H * W

_N_CORES = 8


def _fwd_one(x, w_q, w_k, w_v, w_ck, w_cv, w_out, b_out):
    # x: [C, H, W] — one batch element, one core
    xf = x.reshape(C, N)                      # [C, N]
    q = jnp.dot(w_q, xf)                      # [C, N]
    k = jnp.dot(w_k, xf)
    v = jnp.dot(w_v, xf)

    # depthwise pool, kernel=stride=R (non-overlapping blocks)
    def pool(t, w):
        blocks = t.reshape(C, H // R, R, W // R, R)
        return jnp.einsum('cirjs,crs->cij', blocks, w)

    k_s = pool(k.reshape(C, H, W), w_ck)      # [C, H/R, W/R]
    v_s = pool(v.reshape(C, H, W), w_cv)
    n = (H // R) * (W // R)

    qh = q.reshape(HEADS, DIM, N)
    kh = k_s.reshape(HEADS, DIM, n)
    vh = v_s.reshape(HEADS, DIM, n)

    # stage 1: full queries vs compressed keys
    attn = jax.nn.softmax(jnp.einsum('hdi,hdj->hij', qh, kh) * SCALE, axis=-1)
    token_score = attn.sum(axis=1)            # [HEADS, n]
    top_k = max(1, int(n * TOPK_FRAC))
    _, idx = jax.lax.top_k(token_score, top_k)  # [HEADS, top_k]

    idx_exp = jnp.broadcast_to(idx[:, None, :], (HEADS, DIM, top_k))
    k_top = jnp.take_along_axis(kh, idx_exp, axis=2)
    v_top = jnp.take_along_axis(vh, idx_exp, axis=2)

    # stage 2: attend only to selected compressed tokens
    attn_full = jax.nn.softmax(jnp.einsum('hdi,hdj->hij', qh, k_top) * SCALE, axis=-1)
    out = jnp.einsum('hij,hdj->hdi', attn_full, v_top)   # [HEADS, DIM, N]
    out = out.reshape(C, N)
    return (jnp.dot(w_out, out) + b_out[:, None]).reshape(C, H, W)


_pm = None


def _get_pm():
    global _pm
    if _pm is None:
        devs = jax.devices()
        if len(devs) >= _N_CORES:
            _pm = jax.pmap(
                _fwd_one,
                in_axes=(0, None, None, None, None, None, None, None),
                devices=devs[:_N_CORES],
            )
        else:
            # fallback: single-device batched execution
            _pm = jax.jit(jax.vmap(
                _fwd_one,
                in_axes=(0, None, None, None, None, None, None, None),
            ))
    return _pm


def kernel(x, w_q, w_k, w_v, w_ck, w_cv, w_out, b_out):
    pm = _get_pm()
    args = [
        np.ascontiguousarray(np.asarray(a, dtype=np.float32))
        for a in (x, w_q, w_k, w_v, w_ck, w_cv, w_out, b_out)
    ]
    out = pm(*args)
    return np.asarray(out).astype(np.float32)
